# revision 15
# baseline (speedup 1.0000x reference)
"""Trainium2 Bass kernel for BarlowTwinsLoss (nn_BarlowTwinsLoss_11038065951192).

Full inputs: e_q, tau [16384, 2048] f32. Output: scalar f32 loss.

Strategy (data-parallel over the batch axis, 8 NeuronCores):
  - host quantizes e_q/tau to bf16 (the math below is insensitive to the
    0.4% input quantization noise: it cancels in the correlation) and
    row-shards them: each core gets a [2048, 2048] bf16 shard of each.
    bf16 I/O halves HBM traffic vs f32 (memory-bound regime) and removes
    the on-device f32->bf16 conversion passes entirely.
  - per 128-row tile the DVE computes e*e, t*t, e*t in bf16; the PE
    reduces all 5 statistics streams (e, t, e2, t2, et) over the batch
    rows via ones-vector matmuls accumulating in PSUM (outputs packed on
    PSUM partitions {0,32,64}, 2 stats x 4 512-col chunks per partition).
  - the accumulated [65, 4096] f32 partial sums are staged PSUM->SBUF
    (DMA cannot read PSUM) and DMA'd out as a [3, 4096] tensor per core.
  - the host sums the 8 cores' partial stats (the unshard step) and runs
    the D-length epilogue (mean/std/diag-corr/loss) in float64 - O(D)
    work, negligible next to the O(N*D) on-device reduction.

Hardware pitfalls baked into this design (probed on silicon by an earlier
session; the simulator accepts all of them but hardware does not):
  - DVE tensor_tensor with f32 inputs and bf16 output produces garbage ->
    keep DVE ops all-bf16
  - InstTensorTensorReduce crashes the exec unit -> tensor_mul + matmul
  - ACT reading bf16 input crashes the exec unit -> ACT only touches the
    f32 PSUM->SBUF staging copy
  - DMA cannot read PSUM -> stage through SBUF with a compute-engine copy
"""

import numpy as np

N_FULL = 16384
D = 2048
N_CORES = 8
N_SHARD = N_FULL // N_CORES  # 2048 rows per core
P = 128
N_TILES = N_SHARD // P  # 16
CHUNK = 512
N_CHUNKS = D // CHUNK  # 4
EPS = 1e-9

_CACHE = {}


def _build_nc(
    repeat=1,
    loop=None,
    dma_t_engine="sync",
    s1="pe",
    io_bufs=4,
    dma_batch=1,
    mm_chunk=CHUNK,
):
    import contextlib

    import concourse.bacc as bacc
    import concourse.tile as tile
    from concourse import mybir

    f32 = mybir.dt.float32
    bf16 = mybir.dt.bfloat16

    nc = bacc.Bacc(
        "TRN2",
        target_bir_lowering=False,
        debug=False,
        enable_asserts=False,
        num_devices=1,
    )
    eq_d = nc.dram_tensor("e_q", [N_SHARD, D], bf16, kind="ExternalInput")
    ta_d = nc.dram_tensor("tau", [N_SHARD, D], bf16, kind="ExternalInput")
    out_d = nc.dram_tensor("out", [3, 2 * N_CHUNKS * CHUNK], f32, kind="ExternalOutput")

    t_dma = nc.scalar if dma_t_engine == "scalar" else nc.sync

    with tile.TileContext(nc) as tc:
        with (
            tc.tile_pool(name="io", bufs=io_bufs) as io,
            tc.tile_pool(name="bfp", bufs=3) as bfp,
            tc.tile_pool(name="misc", bufs=1) as misc,
            tc.tile_pool(name="ep", bufs=1) as ep,
            tc.tile_pool(name="psp", bufs=1, space="PSUM") as psp,
        ):
            ones_bf = misc.tile([P, 1], bf16)
            nc.gpsimd.memset(ones_bf[:], 1.0)

            # 5 stats accumulate in PSUM: stat s -> partition 32*(s//2),
            # columns [(s%2)*2048 + c*512, ...). Rows {0,32,64} are the only
            # partitions written (matmul M=1 targets must start on a
            # 32-partition group); the unwritten upper half of row 64 is
            # never consumed by the host.
            psum_stats = psp.tile([65, 2 * N_CHUNKS * CHUNK], f32, tag="stats")

            for _rep in range(repeat):
                loop_cm = (
                    tc.For_i(
                        0,
                        loop,
                        1,
                        hint_engines=(
                            mybir.EngineType.PE,
                            mybir.EngineType.DVE,
                            mybir.EngineType.Activation,
                            mybir.EngineType.SP,
                        ),
                    )
                    if loop is not None
                    else contextlib.nullcontext()
                )
                with contextlib.ExitStack() as _stack:
                    _stack.enter_context(loop_cm)

                    prev = {}
                    for i in range(N_TILES):
                        b = i % dma_batch
                        if b == 0:
                            eg = io.tile([P, dma_batch, D], bf16, tag="e")
                            tg = io.tile([P, dma_batch, D], bf16, tag="t")
                            rows = slice(i * P, (i + dma_batch) * P)
                            nc.sync.dma_start(
                                eg[:],
                                eq_d[rows].rearrange("(b p) c -> p b c", p=P),
                            )
                            t_dma.dma_start(
                                tg[:],
                                ta_d[rows].rearrange("(b p) c -> p b c", p=P),
                            )
                        e_t = eg[:, b, :]
                        t_t = tg[:, b, :]

                        e2_bf = bfp.tile([P, D], bf16, tag="e2_bf")
                        t2_bf = bfp.tile([P, D], bf16, tag="t2_bf")
                        et_bf = bfp.tile([P, D], bf16, tag="et_bf")
                        nc.vector.tensor_mul(e2_bf[:], e_t, e_t)
                        nc.vector.tensor_mul(t2_bf[:], t_t, t_t)
                        nc.vector.tensor_mul(et_bf[:], e_t, t_t)

                        def _sl(tile, c, b=None):
                            lo, hi = c * mm_chunk, (c + 1) * mm_chunk
                            if b is None:
                                return tile[:, lo:hi]
                            return tile[:, b, lo:hi]

                        if s1 == "pe":
                            srcs = [
                                (0, eg, b, i == 0, i == N_TILES - 1),
                                (1, tg, b, i == 0, i == N_TILES - 1),
                            ]
                        elif s1 == "quad":
                            # 4-tile Pool add tree per S1 stat: stream one
                            # quad-sum per 4 tiles (quarter PE S1 work)
                            srcs = []
                            if i % 4 == 0:
                                prev = {"e": e_t, "t": t_t}
                            elif i % 4 == 1:
                                qa = {}
                                for nmx, cur in (("e", e_t), ("t", t_t)):
                                    a = bfp.tile([P, D], bf16, tag=f"qa{nmx}")
                                    nc.gpsimd.tensor_add(a[:], prev[nmx], cur)
                                    qa[nmx] = a
                            elif i % 4 == 2:
                                prev = {"e": e_t, "t": t_t}
                            else:
                                for s, nmx, cur in ((0, "e", e_t), (1, "t", t_t)):
                                    bb = bfp.tile([P, D], bf16, tag=f"qb{nmx}")
                                    nc.gpsimd.tensor_add(bb[:], prev[nmx], cur)
                                    q = bfp.tile([P, D], bf16, tag=f"qq{nmx}")
                                    nc.gpsimd.tensor_add(q[:], qa[nmx][:], bb[:])
                                    srcs.append(
                                        (s, q, None, i == 3, i == N_TILES - 1)
                                    )
                        elif i % 2 == 0:
                            # pair-add: buffer this tile, stream the pair sum
                            # on the odd tile (halves the PE S1 work)
                            prev = {"e": e_t, "t": t_t}
                            srcs = []
                        else:
                            se = bfp.tile([P, D], bf16, tag="se")
                            st_ = bfp.tile([P, D], bf16, tag="st")
                            # "pair2": most pair-adds go to the (otherwise
                            # idle) Pool engine, the rest stay on DVE, so
                            # neither engine becomes the new bottleneck.
                            eng1 = eng2 = nc.vector
                            if s1 == "pair2":
                                eng1 = nc.gpsimd
                                eng2 = nc.gpsimd if i < 12 else nc.vector
                            elif s1 == "pair3":  # all pair-adds on Pool
                                eng1 = eng2 = nc.gpsimd
                            eng1.tensor_add(se[:], prev["e"], e_t)
                            eng2.tensor_add(st_[:], prev["t"], t_t)
                            srcs = [
                                (0, se, None, i == 1, i == N_TILES - 1),
                                (1, st_, None, i == 1, i == N_TILES - 1),
                            ]
                        srcs += [
                            (2, e2_bf, None, i == 0, i == N_TILES - 1),
                            (3, t2_bf, None, i == 0, i == N_TILES - 1),
                            (4, et_bf, None, i == 0, i == N_TILES - 1),
                        ]

                        n_mm = D // mm_chunk
                        for s, src, sb, st0, st1 in srcs:
                            g, sl = divmod(s, 2)
                            for c in range(n_mm):
                                col = sl * D + c * mm_chunk
                                nc.tensor.matmul(
                                    psum_stats[
                                        32 * g : 32 * g + 1, col : col + mm_chunk
                                    ],
                                    ones_bf[:, 0:1],
                                    _sl(src, c, sb),
                                    start=st0,
                                    stop=st1,
                                )

                    # PSUM -> SBUF staging (DMA and GPSIMD cannot read PSUM,
                    # BIR-verified). ACT is otherwise idle (it cannot read
                    # bf16), so it takes most of the copy; DVE the rest.
                    sb_stats = ep.tile(
                        [65, 2 * N_CHUNKS * CHUNK], f32, tag="sb_stats"
                    )
                    cut = 5 * CHUNK
                    nc.scalar.copy(
                        sb_stats[:, :cut], psum_stats[:, :cut]
                    )
                    nc.vector.tensor_copy(
                        sb_stats[:, cut:], psum_stats[:, cut:]
                    )

                    # partial stats out: partitions {0,32,64} of the staged
                    # tile, split column-wise into four DMAs over two DGE
                    # engines so they spread across DMA queues.
                    q = N_CHUNKS * CHUNK // 2
                    for k, eng in enumerate((nc.sync, nc.scalar, nc.sync, nc.scalar)):
                        eng.dma_start(
                            out_d[:, k * q : (k + 1) * q],
                            sb_stats[0:65:32, k * q : (k + 1) * q],
                        )

    nc.compile()
    return nc


class _Exec:
    """Cached PJRT executable for the SPMD kernel (mirrors
    concourse.bass2jax.run_bass_via_pjrt's multi-core branch, but keeps the
    jitted callable so repeat invocations don't recompile)."""

    def __init__(self, nc):
        import jax
        from jax.experimental.shard_map import shard_map
        from jax.sharding import Mesh, PartitionSpec

        from concourse import bass2jax, mybir

        bass2jax.install_neuronx_cc_hook()
        self.nc = nc
        partition_name = (
            nc.partition_id_tensor.name if nc.partition_id_tensor else None
        )

        in_names, out_names, out_avals, zero_outs = [], [], [], []
        for alloc in nc.m.functions[0].allocations:
            if not isinstance(alloc, mybir.MemoryLocationSet):
                continue
            assert alloc.memorylocations
            name = alloc.memorylocations[0].name
            if alloc.kind == "ExternalInput":
                if name != partition_name:
                    in_names.append(name)
            elif alloc.kind == "ExternalOutput":
                shape = tuple(alloc.tensor_shape)
                dtype = mybir.dt.np(alloc.dtype)
                out_names.append(name)
                out_avals.append(jax.core.ShapedArray(shape, dtype))
                zero_outs.append(np.zeros(shape, dtype))

        self.in_names = list(in_names)
        self.out_names = list(out_names)
        self.out_avals = out_avals
        self.zero_outs = zero_outs
        n_params = len(in_names)
        n_outs = len(out_names)

        in_names_full = list(in_names) + list(out_names)
        if partition_name is not None:
            in_names_full.append(partition_name)

        def _body(*args):
            operands = list(args)
            if partition_name is not None:
                operands.append(bass2jax.partition_id_tensor())
            outs = bass2jax._bass_exec_p.bind(
                *operands,
                out_avals=tuple(out_avals),
                in_names=tuple(in_names_full),
                out_names=tuple(out_names),
                lowering_input_output_aliases=(),
                sim_require_finite=False,
                sim_require_nnan=False,
                nc=nc,
            )
            return tuple(outs)

        devices = jax.devices()[:N_CORES]
        assert len(devices) == N_CORES, f"need {N_CORES} devices, got {len(devices)}"
        self.mesh = Mesh(np.asarray(devices), ("core",))
        in_specs = (PartitionSpec("core"),) * (n_params + n_outs)
        out_specs = (PartitionSpec("core"),) * n_outs
        donate = tuple(range(n_params, n_params + n_outs))
        self.sharded = jax.jit(
            shard_map(
                _body,
                mesh=self.mesh,
                in_specs=in_specs,
                out_specs=out_specs,
                check_rep=False,
            ),
            donate_argnums=donate,
            keep_unused=True,
        )

    def concat_zeros(self):
        return [
            np.zeros((N_CORES * z.shape[0], *z.shape[1:]), z.dtype)
            for z in self.zero_outs
        ]

    def run(self, in_map):
        """in_map: name -> full (already concat-along-axis0) array."""
        ins = [in_map[name] for name in self.in_names]
        outs = self.sharded(*ins, *self.concat_zeros())
        return {
            name: np.asarray(outs[i]).reshape(
                N_CORES, *self.out_avals[i].shape
            )
            for i, name in enumerate(self.out_names)
        }


def _get_exec(repeat=1, **kw):
    key = ("exec", repeat, tuple(sorted(kw.items())))
    if key not in _CACHE:
        _CACHE[key] = _Exec(_build_nc(repeat, **kw))
    return _CACHE[key]


def prep_inputs(e_q, tau):
    """Quantize full f32 inputs to the bf16 layout the device consumes."""
    import ml_dtypes

    e_q = np.asarray(e_q, dtype=np.float32).astype(ml_dtypes.bfloat16)
    tau = np.asarray(tau, dtype=np.float32).astype(ml_dtypes.bfloat16)
    return {"e_q": np.ascontiguousarray(e_q), "tau": np.ascontiguousarray(tau)}


def finalize(out8):
    """Host epilogue: combine the 8 cores' partial sums ([8, 3, 4096]) and
    evaluate the D-length loss formula in float64."""
    st = out8.astype(np.float64).sum(axis=0)  # [3, 4096]
    s1e, s1t = st[0, :D], st[0, D:]
    s2e, s2t = st[1, :D], st[1, D:]
    set_ = st[2, :D]
    n = float(N_FULL)
    # sum((x-mean)^2) = S2 - S1^2/N ; std = max(sqrt(./(N-1)), eps)
    var_e = (s2e - s1e * s1e / n) / (n - 1.0)
    var_t = (s2t - s1t * s1t / n) / (n - 1.0)
    std_e = np.maximum(np.sqrt(np.maximum(var_e, 0.0)), EPS)
    std_t = np.maximum(np.sqrt(np.maximum(var_t, 0.0)), EPS)
    cov = set_ - s1e * s1t / n
    c = cov / (std_e * std_t) / (n + EPS)
    c = np.clip(c, -1.0 + EPS, 1.0 - EPS)
    loss = np.square(1.0 - c).sum()
    return np.asarray(loss, dtype=np.float32)


def kernel(e_q, tau):
    assert np.asarray(e_q).shape == (N_FULL, D)
    assert np.asarray(tau).shape == (N_FULL, D)
    ex = _get_exec()
    # row-sharding across cores: the concatenation of the 8 shards along
    # axis 0 is just the full array, so pass it through unchanged.
    outs = ex.run(prep_inputs(e_q, tau))
    return finalize(outs["out"])
